# revision 1
# baseline (speedup 1.0000x reference)
"""Trainium2 Bass kernel for BaselineParameterizedPool2D.

Reference op: 3x3/stride-2/pad-1 max pool over xs [16,64,256,256] where each
of the 9 taps gets a per-(tap,channel) bias h[0,k,c] added before the max;
returns (pooled f32, argmax-tap-index int32), both [16,64,128,128].

Distribution: data-parallel over batch — 8 cores x 2 batches each.
Per-core layout: partitions = (b_local, c) = 2*64 = 128; free dim = spatial.

Per chunk of R=8 output rows (pipelined across all five engines; the real
Pool engine only accepts ARITH tensor ops — no max/is_ge/stt, verified
against neuronxcc — so engine assignment works around that):
  - DMA 16 NEW input rows into a round-robin SBUF tile (col 0 = -10 left
    pad); the boundary row shared with the previous chunk is copied from
    the previous tile by ScalarE instead of re-fetched (-6% input DMA).
  - Prefix-max chain MM[:, s] = max over slots 0..s of (tap + bias), slot s
    holding tap 8-s: ScalarE builds slot 0 (Identity + per-partition bias),
    DVE runs the 8 fused scalar_tensor_tensor (add+max) folds. MM[:, 8] = m
    is the pooled output, written back as bf16 by a GPSIMD tensor_scalar
    copy (2^-9 rel err << the 2e-2 gate; host casts to f32).
  - Provenance via exact subtract-and-sign: D = MM[0:8] - m as GPSIMD
    subtracts (f32 arithmetic, bf16 out — sign-exact since any nonzero f32
    gap is far above bf16's min normal, and m - m = +0.0), split 6+2 so
    the sign stage starts before the second half finishes. Slot 0 becomes
    a {0,1} indicator on DVE (tensor_scalar is_ge 0, 4x perf mode); slots
    1..7 become {-1,0} via in-place ScalarE Sign ops split to match.
  - Count = argmax tap index: the 8 D-slots are summed ON THE TENSOR
    ENGINE with 8 accumulating fp8-identity matmuls per 4-row group into a
    PSUM bank (f32 accumulate — exact for these small integers); ScalarE
    reads PSUM -> int8 prov with a +7 bias column (host casts to int32).
  - The last chunk drains entirely on DVE (subtract, is_ge, bf16 pairwise
    tree) to shorten the un-overlappable tail.
Engine busy per full chunk: DVE ~9.4us (8 folds + indicator), ScalarE
~9.0us (build + boundary copy + Sign + PSUM reads), GPSIMD ~7.9us
(subtract + pooled copy), PE ~4.5us, DMA ~7.3us.
"""

import numpy as np

import concourse.bacc as bacc
import concourse.bass as bass
import concourse.mybir as mybir
from concourse.tile import TileContext

F32 = mybir.dt.float32
BF16 = mybir.dt.bfloat16
I8 = mybir.dt.int8
FP8 = mybir.dt.float8e4

B = 16          # full batch
NCORES = 8
B_LOC = B // NCORES   # 2
C = 64
H = 256
W = 256
HO = 128
WO = 128
KS = 3
PAD = -10.0

R = 8                   # output rows per full chunk
NCHUNK = HO // R        # full chunks per core
G = 4                   # output rows per PSUM count group (512 f32 = 1 bank)


def emit(nc: bass.Bass, nchunk: int = NCHUNK):
    xs_d = nc.dram_tensor("xs", [B_LOC, C, H, W], F32, kind="ExternalInput")
    h_d = nc.dram_tensor("h", [1, KS * KS, C], F32, kind="ExternalInput")
    pooled_d = nc.dram_tensor("pooled", [B_LOC, C, HO, WO], BF16,
                              kind="ExternalOutput")
    prov_d = nc.dram_tensor("prov", [B_LOC, C, HO, WO], I8,
                            kind="ExternalOutput")
    ident_d = nc.dram_tensor("ident", [128, 128], FP8, kind="ExternalInput")

    xs_f = xs_d.ap().rearrange("b c h w -> (b c) h w")          # [128, 256, 256]
    pooled_f = pooled_d.ap().rearrange("b c h w -> (b c) h w")  # [128, 128, 128]
    prov_f = prov_d.ap().rearrange("b c h w -> (b c) h w")

    with TileContext(nc) as tc:
        with (
            tc.tile_pool(name="const", bufs=1) as constp,
            tc.tile_pool(name="io", bufs=3) as iop,
            tc.tile_pool(name="work", bufs=2) as workp,
            tc.tile_pool(name="psum", bufs=3, space="PSUM") as psump,
        ):
            # h_sb[p, k] = h[0, k, p % 64] : per-partition bias columns
            h_sb = constp.tile([128, KS * KS], F32)
            h_src = h_d.ap()[0].transpose([1, 0])   # [64, 9]
            nc.sync.dma_start(h_sb[0:64, :], h_src)
            nc.sync.dma_start(h_sb[64:128, :], h_src)

            # fp8 identity for the PE count matmuls: ident[p, j] = (j == p),
            # loaded as a constant auxiliary input (affine_select does not
            # pass the neuronxcc Pool engine check)
            ident = constp.tile([128, 128], FP8)
            nc.sync.dma_start(ident[:], ident_d.ap())

            # per-partition bias columns for the PSUM->int8 prov read:
            # +7 for the sign path ({-1,0} slots 1..7), 0 for the all-DVE
            # indicator path ({0,1} all slots, used in the tail)
            bias7 = constp.tile([128, 1], F32)
            nc.vector.memset(bias7[:], 7.0)
            bias0 = constp.tile([128, 1], F32)
            nc.vector.memset(bias0[:], 0.0)

            # persistent round-robin input tiles: pad col/row memset once
            NR = 2 * R + 2
            xin_bufs = [constp.tile([128, NR, 258], F32, name=f"xin{i}")
                        for i in range(3)]
            for xb in xin_bufs:
                nc.gpsimd.memset(xb[:, :, 0:1], PAD)
            nc.gpsimd.memset(xin_bufs[0][:, 0:1, :], PAD)  # row -1 (chunk 0)

            # Chunk schedule: small chunks at the start (compute ramps while
            # the first DMA lands) and at the end (shorter un-overlappable
            # cmp->count->dma drain), full R=8 chunks between.
            total_rows = nchunk * R
            sched = [(0, 4), (4, 4)]
            sched += [(i0, R) for i0 in range(8, total_rows - 8, R)]
            if total_rows >= 16:
                sched += [(total_rows - 8, 4), (total_rows - 4, 2),
                          (total_rows - 2, 2)]

            # warm the ACT function table so the first build doesn't pay
            # the 1283 ns table load on the critical path (source is a local
            # memset tile — must not wait on any DMA)
            warm = constp.tile([128, 1], F32)
            nc.vector.memset(warm[:], 0.0)
            nc.scalar.activation(warm[:], warm[:],
                                 mybir.ActivationFunctionType.Identity)

            # collapse all setup waits so per-chunk ops carry few sync slots
            tc.strict_bb_all_engine_barrier()

            def flush(i0, RC, MM, last=False, dve_inds=False):
                # Post-chain work for a finished chunk. The real Pool engine
                # only supports ARITH tensor ops (add/sub/mult) — no max or
                # is_ge — so provenance uses the exact subtract-and-sign
                # trick: D_s = MM_s - m in f32 arithmetic with bf16 output
                # has an exact sign (any nonzero f32 gap is far above bf16's
                # min normal; m - m = +0.0). One big GPSIMD subtract, then:
                #   slot 0:  DVE tensor_scalar (D >= 0) -> {1, 0}  (2x mode)
                #   slots 1..7: ScalarE Sign in place -> {-1, 0}
                # PSUM-sum of all 8 via identity matmuls then gives
                # prov = PSUM + 7, folded into the psum-read bias column.
                D = workp.tile([128, KS * KS - 1, RC, WO], BF16, tag="D",
                               bufs=2)
                m_b = MM[:, 8:9].broadcast_to([128, KS * KS - 1, RC, WO])
                if last:
                    # Tail drain: keep everything on DVE (no cross-engine
                    # hops after the last chain): subtract, is_ge-0 at 4x,
                    # bf16 pairwise-sum tree (exact for counts <= 8).
                    nc.vector.tensor_tensor(D[:], MM[:, 0:KS * KS - 1], m_b,
                                            op=mybir.AluOpType.subtract)
                    nc.vector.tensor_scalar(D[:], D[:], 0.0, None,
                                            op0=mybir.AluOpType.is_ge)
                    nc.vector.tensor_tensor(D[:, 0:4], D[:, 0:4], D[:, 4:8],
                                            op=mybir.AluOpType.add)
                    nc.vector.tensor_tensor(D[:, 0:2], D[:, 0:2], D[:, 2:4],
                                            op=mybir.AluOpType.add)
                    nc.vector.tensor_tensor(D[:, 0], D[:, 0], D[:, 1],
                                            op=mybir.AluOpType.add)
                    prov_t = iop.tile([128, RC, WO], I8, tag="prov", bufs=3)
                    nc.vector.tensor_scalar(prov_t[:], D[:, 0], 0.0, None,
                                            op0=mybir.AluOpType.add)
                    pooled_t = iop.tile([128, RC, WO], BF16, tag="pooled",
                                        bufs=3)
                    nc.gpsimd.tensor_scalar(pooled_t[:], MM[:, KS * KS - 1],
                                            0.0, None,
                                            op0=mybir.AluOpType.add)
                    nc.sync.dma_start(pooled_f[:, i0:i0 + RC, :], pooled_t[:])
                    nc.sync.dma_start(prov_f[:, i0:i0 + RC, :], prov_t[:])
                    return
                m_ba = MM[:, 8:9].broadcast_to([128, 6, RC, WO])
                nc.gpsimd.tensor_tensor(D[:, 0:6], MM[:, 0:6], m_ba,
                                        op=mybir.AluOpType.subtract)
                nc.gpsimd.tensor_tensor(D[:, 6:KS * KS - 1],
                                        MM[:, 6:KS * KS - 1],
                                        m_b[:, 6:KS * KS - 1],
                                        op=mybir.AluOpType.subtract)
                if dve_inds:
                    # tail: DVE is draining and idle — take all 8 indicator
                    # slots there ({0,1} via is_ge at 4x, bias 0)
                    nc.vector.tensor_scalar(D[:], D[:], 0.0, None,
                                            op0=mybir.AluOpType.is_ge)
                else:
                    nc.vector.tensor_scalar(D[:, 0], D[:, 0], 0.0, None,
                                            op0=mybir.AluOpType.is_ge)
                    nc.scalar.activation(D[:, 1:6], D[:, 1:6],
                                         mybir.ActivationFunctionType.Sign)
                    nc.scalar.activation(D[:, 6:KS * KS - 1],
                                         D[:, 6:KS * KS - 1],
                                         mybir.ActivationFunctionType.Sign)

                if True:
                    prov_t = iop.tile([128, RC, WO], I8, tag="prov", bufs=3)
                    for g0 in range(0, RC, G):
                        gr = min(G, RC - g0)
                        cnt = psump.tile([128, gr, WO], F32, tag="cnt",
                                         bufs=3)
                        for s in range(KS * KS - 1):
                            nc.tensor.matmul(
                                cnt[:], ident[:], D[:, s, g0:g0 + gr, :],
                                start=(s == 0), stop=(s == KS * KS - 2))
                        nc.scalar.activation(
                            prov_t[:, g0:g0 + gr], cnt[:],
                            mybir.ActivationFunctionType.Identity,
                            bias=(bias0 if dve_inds else bias7)[:, 0:1],
                            scale=1.0)

                # pooled bf16 copy on GPSIMD (it has slack; ts-arith is legal)
                pooled_t = iop.tile([128, RC, WO], BF16, tag="pooled", bufs=3)
                nc.gpsimd.tensor_scalar(pooled_t[:], MM[:, KS * KS - 1], 0.0,
                                        None, op0=mybir.AluOpType.add)

                nc.sync.dma_start(pooled_f[:, i0:i0 + RC, :], pooled_t[:])
                nc.sync.dma_start(prov_f[:, i0:i0 + RC, :], prov_t[:])

            pending = None
            prev = None           # (tile, RC) of the previous chunk
            for ch, (i0, RC) in enumerate(sched):
                xin = xin_bufs[ch % len(xin_bufs)]
                # Tile row i holds input row 2*i0-1+i. DMA fetches rows
                # 1..2*RC only; row 0 (the boundary row, shared with the
                # previous chunk) is copied from the previous tile's last
                # fetched row by the otherwise-idle ScalarE — saves 6% of
                # input DMA. Chunk 0's row 0 is the PAD row (memset above).
                nc.sync.dma_start(xin[:, 1:2 * RC + 1, 1:257],
                                  xs_f[:, 2 * i0:2 * i0 + 2 * RC, :])
                if i0 != 0:
                    pxin, pRC = prev
                    nc.scalar.activation(
                        xin[:, 0:1, 1:257], pxin[:, 2 * pRC:2 * pRC + 1, 1:257],
                        mybir.ActivationFunctionType.Identity)
                prev = (xin, RC)

                # Prefix-max chain: MM[:, s] = max over slots 0..s of
                # (tap + bias), slot s = tap 8-s; MM[:, 8] = m is the max.
                # ScalarE builds slot 0, GPSIMD folds slot 1, DVE runs slots
                # 2..8 — DVE (7 stt) and GPSIMD (1 stt + 8 cmp) both carry
                # ~7.9 us per full chunk.
                MM = workp.tile([128, KS * KS, RC, WO], F32, tag="MM", bufs=3)
                for s in range(KS * KS):
                    k = 8 - s
                    di, dj = divmod(k, 3)
                    src = xin[:, di:di + 2 * RC - 1:2, dj:dj + 2 * WO:2]
                    if s == 0:
                        nc.scalar.activation(
                            MM[:, 0], src,
                            mybir.ActivationFunctionType.Identity,
                            bias=h_sb[:, k:k + 1], scale=1.0)
                    else:
                        nc.vector.scalar_tensor_tensor(
                            MM[:, s], src, h_sb[:, k:k + 1], MM[:, s - 1],
                            op0=mybir.AluOpType.add, op1=mybir.AluOpType.max)

                if pending is not None:
                    flush(*pending, dve_inds=(ch >= len(sched) - 4))
                pending = (i0, RC, MM)
            flush(*pending, last=True)
    return nc


def build_nc(nchunk: int = NCHUNK, compile: bool = True):
    nc = bacc.Bacc("TRN2", target_bir_lowering=False, debug=False)
    emit(nc, nchunk=nchunk)
    if compile:
        nc.compile()
    return nc


_NC_CACHE = []


def kernel(xs: np.ndarray, h: np.ndarray):
    from concourse.bass_utils import run_bass_kernel_spmd

    xs = np.ascontiguousarray(xs, dtype=np.float32)
    h = np.ascontiguousarray(h, dtype=np.float32)
    if not _NC_CACHE:
        _NC_CACHE.append(build_nc())
    nc = _NC_CACHE[0]
    import ml_dtypes
    ident = np.eye(128, dtype=np.float32).astype(ml_dtypes.float8_e4m3)
    in_maps = [
        {"xs": np.ascontiguousarray(xs[i * B_LOC:(i + 1) * B_LOC]), "h": h,
         "ident": ident}
        for i in range(NCORES)
    ]
    res = run_bass_kernel_spmd(nc, in_maps, core_ids=list(range(NCORES)))
    pooled = np.concatenate(
        [np.asarray(r["pooled"]).astype(np.float32) for r in res.results],
        axis=0)
    prov = np.concatenate(
        [np.asarray(r["prov"]).astype(np.int32) for r in res.results], axis=0)
    return pooled, prov

